# revision 6
# baseline (speedup 1.0000x reference)
"""Trainium2 Bass kernel for nn_Disp_61125974557155 (v3).

Computes: trilinear upsample of a cost volume [B,1,48,64,128] ->
[B,193,256,512] (align_corners=False, edge-replicated), softmin over
disparity, disparity regression -> [B,256,512].

v3 structural changes vs v2 (103us) / baseline (105.6us):
  - Non-flipped stats: stationary = rmat [K,2] (2-column weight loads),
    moving = the e tiles. Kills v2's 512 x ~110ns stats LDWEIGHTS chain
    (54.8us of serialized weight-port traffic at the fixed 1.2GHz LDW
    clock) entirely.
  - Stats run as 4-way column-group-concurrent quads (tile_position
    col slots 0/32/64/96, one 4-row PSUM bank per quad): 4 independent
    moving streams overlap on separate XBUSes, so 4 rows' S0/S1 cost
    ~1 stream wall instead of 4.
  - The quad bank is drained by one DVE copy (psum->sbuf image) and two
    partition-stride-32 gather DMAs (sbuf->sbuf, validated on HW) into a
    row-major [128, 512] stats tile: S0 rows at partitions o, S1 at 64+o.
  - Tail is transpose-free: the x47 plane (d'191,192 == x[47] exactly)
    joins via two [64,512] STT adds in the same row-major layout, then
    approx-reciprocal, multiply, one shuffled copy to w'-order, and four
    strided output DMAs (one per r-phase).
  - All-fp16 operands everywhere on PE (precision sim 2.2e-4 rel err).
"""

import os
import numpy as np
from contextlib import ExitStack

DBG_NO_GATHER = os.environ.get("V3_NO_GATHER") == "1"
DBG_NO_QUADS = os.environ.get("V3_NO_QUADS") == "1"
DBG_NO_X47 = os.environ.get("V3_NO_X47") == "1"
DBG_NO_C1Q = os.environ.get("V3_NO_C1Q") == "1"
DBG_NO_CLR = os.environ.get("V3_NO_CLR") == "1"
DBG_NO_IMG = os.environ.get("V3_NO_IMG") == "1"



import concourse.bass as bass
import concourse.bacc as bacc
import concourse.tile as tile
from concourse import mybir
from concourse.bass_utils import run_bass_kernel_spmd

F32 = mybir.dt.float32
F16 = mybir.dt.float16

MAXDISP = 192
DP = MAXDISP + 1       # 193 disparities
DM = 191               # matmul disparities (d'191,192 == x47 exactly)
KD = 48                # low-res D
KP = KD + 2            # padded k' (edge-replicated)
NCORES = 8
WH = (0.625, 0.875, 0.125, 0.375)   # H lerp fracs per r = h' % 4
NROW = 19                            # h-rows in dup-packed shard
ROW_GROUPS = ((0, 1), (1, 1), (2, 2), (4, 4), (8, 4), (12, 4), (16, 3))
RV0, RV1 = 2.0, 383.0  # x47-plane stats weights: (1+1, 191+192)


def _build_ad() -> np.ndarray:
    """A_D [191, 50]: D-axis linear upsample matrix on padded k' = k+1."""
    ad = np.zeros((DM, KP), dtype=np.float64)
    for dp in range(DM):
        i = (dp + 0.5) * KD / DP - 0.5
        fl = int(np.floor(i))
        fr = i - fl
        ad[dp, fl + 1] += 1.0 - fr
        ad[dp, fl + 2] += fr
    return ad


def _build_consts():
    ad = _build_ad()                          # [191, 50]
    amat = np.zeros((2 * KP, 4, 192), dtype=np.float64)
    for r in range(4):
        w = np.zeros((2 * KP, 192))
        w[:KP, :DM] = (1.0 - WH[r]) * ad.T
        w[KP:, :DM] = WH[r] * ad.T
        # cols: [0:128] = c0, [128:191] = c1, [191] = zero pad
        amat[:, r, 0:128] = w[:, 0:128]
        amat[:, r, 128:191] = w[:, 128:191]
    # cols 0,1: c0 (S0, S1); cols 2,3: c1 half0 rvec zero-masked above 62;
    # cols 4,5: c1 half1 rvec zero-masked below 64 (lets every stats matmul
    # contract K=128 at row_grp 0 -- tile_position (64, 32c) faults on HW);
    # cols 6,7: zeros (bank-clear matmul weight).
    rmat = np.zeros((128, 8), dtype=np.float64)
    rmat[:, 0] = 1.0
    rmat[:, 1] = np.arange(128)
    for p in range(63):
        rmat[p, 2] = 1.0
        rmat[p, 3] = 128 + p
        rmat[64 + p, 4] = 1.0
        rmat[64 + p, 5] = 128 + p
    # hmat columns in stats processing order o = 16r + t  (row j = 4t+r)
    hmat = np.zeros((18, 64), dtype=np.float64)
    for o in range(64):
        r, t = divmod(o, 16)
        l = t + (1 if r >= 2 else 0)
        hmat[l, o] += 1.0 - WH[r]
        hmat[l + 1, o] += WH[r]
    f16 = mybir.dt.np(F16)
    return (
        np.ascontiguousarray(amat.reshape(2 * KP, 4 * 192)).astype(f16),
        rmat.astype(f16),
        hmat.astype(f16),
    )


def _build_nc() -> bass.Bass:
    nc = bacc.Bacc()
    xsd = nc.declare_dram_parameter("xsd", [2 * KP, NROW * 130], F16, isOutput=False)
    x47 = nc.declare_dram_parameter("x47", [18, 130], F16, isOutput=False)
    amat = nc.declare_dram_parameter("amat", [2 * KP, 4 * 192], F16, isOutput=False)
    rmat = nc.declare_dram_parameter("rmat", [128, 8], F16, isOutput=False)
    hmat = nc.declare_dram_parameter("hmat", [18, 64], F16, isOutput=False)
    outp = nc.declare_dram_parameter("out", [64, 512], F32, isOutput=True)

    xsd_v = xsd.rearrange("p (h w) -> p h w", h=NROW)
    amat_v = amat.rearrange("p (r d) -> p r d", r=4)

    mult = mybir.AluOpType.mult
    add = mybir.AluOpType.add
    exp_fn = mybir.ActivationFunctionType.Exp

    with ExitStack() as ctx:
        tc = ctx.enter_context(tile.TileContext(nc))
        singles = ctx.enter_context(tc.tile_pool(name="singles", bufs=1))
        tmp_pool = ctx.enter_context(tc.tile_pool(name="tmp", bufs=4))
        epool = ctx.enter_context(tc.tile_pool(name="epool", bufs=6))
        imgp = ctx.enter_context(tc.tile_pool(name="imgp", bufs=3))
        fin = ctx.enter_context(tc.tile_pool(name="fin", bufs=1))
        pvol = ctx.enter_context(tc.tile_pool(name="pvol", bufs=2, space="PSUM"))
        pstat = ctx.enter_context(tc.tile_pool(name="pstat", bufs=2, space="PSUM"))

        # ---- input loads: xsd first (gates the lerp chain) on the sync
        # HWDGE queue; amat first among the gpsimd SWDGE consts ----
        s_xsd = []
        for g, (g0, gn) in enumerate(ROW_GROUPS):
            t_x = singles.tile([2 * KP, gn, 130], F16, tag=f"xsd{g}")
            nc.sync.dma_start(out=t_x, in_=xsd_v[:, g0 : g0 + gn, :])
            s_xsd.append(t_x)
        s_am = singles.tile([2 * KP, 4, 192], F16, tag="am")
        nc.gpsimd.dma_start(out=s_am, in_=amat_v[:, :, :])
        s_rm = singles.tile([128, 8], F16, tag="rm")
        nc.gpsimd.dma_start(out=s_rm, in_=rmat[:, :])
        s_x47 = singles.tile([18, 130], F16, tag="x47")
        nc.gpsimd.dma_start(out=s_x47, in_=x47[:, :])
        s_hm = singles.tile([18, 64], F16, tag="hm")
        nc.gpsimd.dma_start(out=s_hm, in_=hmat[:, :])

        # ---- W-axis 4x lerp at low res, rw-major planes (fp16) ----
        #   rw0 = xs[s+1] + 0.375*d[s]    rw1 = xs[s+1] + 0.125*d[s]
        #   rw2 = xs[s+2] + 0.875*d[s+1]  rw3 = xs[s+2] + 0.625*d[s+1]
        WL = ((0.375, 0, 1), (0.125, 0, 1), (0.875, 1, 2), (0.625, 1, 2))

        def wlerp(src, dst, dtile):
            nc.vector.tensor_sub(dtile, src[..., 0:129], src[..., 1:130])
            for rw, (coef, dc, hc) in enumerate(WL):
                nc.vector.scalar_tensor_tensor(
                    out=dst[..., rw, :],
                    in0=dtile[..., dc : dc + 128],
                    scalar=coef,
                    in1=src[..., hc : hc + 128],
                    op0=mult,
                    op1=add,
                )

        s_xsw = []
        for g, (g0, gn) in enumerate(ROW_GROUPS):
            t_w = singles.tile([2 * KP, gn, 4, 128], F16, tag=f"xsw{g}")
            t_d = tmp_pool.tile([2 * KP, gn, 129], F16, tag="wld")
            wlerp(s_xsd[g], t_w, t_d)
            s_xsw.append(t_w)
            if g == 1:
                s_xw47 = singles.tile([18, 4, 128], F16, tag="xw47")
                t_d47 = tmp_pool.tile([18, 129], F16, tag="wld47")
                wlerp(s_x47, s_xw47, t_d47)

        def xsw_row(l: int) -> bass.AP:
            for g, (g0, gn) in enumerate(ROW_GROUPS):
                if g0 <= l < g0 + gn:
                    return s_xsw[g][:, l - g0, :, :].rearrange("p q s -> p (q s)")
            raise IndexError(l)

        # row-major stats, per-32-row-block base-0 tiles (custom DVE ops
        # and partition-shifted reads are only safe at base partition 0)
        s_st0 = [singles.tile([32, 512], F32, name=f"st0{b}", tag=f"st0{b}") for b in range(2)]
        s_st1 = [singles.tile([32, 512], F32, name=f"st1{b}", tag=f"st1{b}") for b in range(2)]
        s_e192 = singles.tile([64, 512], F32, tag="e192")
        s_e192b = singles.tile([32, 512], F32, tag="e192b")

        # ---- main loop: 4 r-phases x 8 t-pair granules ----
        # granule psum [128, 1536]:
        #   [0:128,    0:512 ] c0 row a      [0:128, 512:1024] c0 row b
        #   [0:64,  1024:1280] c1 row a h0   [64:128, 1024:1280] c1 row a h1
        #   [0:64,  1280:1536] c1 row b h0   [64:128, 1280:1536] c1 row b h1
        def emit_vol(pv, r, l, slot):
            # both c0 matmuls first (shared stationary), then the four c1
            # half-matmuls (shared stationary): one amat LDW alternation per
            # granule instead of two
            lhs0 = s_am[:, r, 0:128]
            lhs1 = s_am[:, r, 128:192]
            rhs0 = xsw_row(l)
            rhs1 = xsw_row(l + 1)
            nc.tensor.matmul(pv[0:128, 0:512], lhs0, rhs0,
                             start=True, stop=True)
            nc.tensor.matmul(pv[0:128, 512:1024], lhs0, rhs1,
                             start=True, stop=True)
            nc.tensor.matmul(pv[0:64, 1024:1280], lhs1, rhs0[:, 0:256],
                             start=True, stop=True)
            nc.tensor.matmul(pv[64:128, 1024:1280], lhs1, rhs0[:, 256:512],
                             start=True, stop=True)
            nc.tensor.matmul(pv[0:64, 1280:1536], lhs1, rhs1[:, 0:256],
                             start=True, stop=True)
            nc.tensor.matmul(pv[64:128, 1280:1536], lhs1, rhs1[:, 256:512],
                             start=True, stop=True)

        # Stats quad: 4 rows (2 granules) -> one PSUM bank, 12 matmuls.
        # c0 x4 at col slots 0/32/64/96 run concurrently (distinct col
        # groups, own XBUS each); then c1 half0 x4 and half1 x4. The quad's
        # first matmul starts the bank's accumulation group; per-element
        # has_written then gives overwrite-where-clear / accumulate.
        def emit_quad(ets, q):
            ps = pstat.tile([128, 512], F32, tag="ps")
            # start=True on each slot's first matmul: measured semantics are
            # per-element (start=True overwrites exactly its own outputs;
            # start=False accumulates onto whatever is there, including
            # stale content from previous NEFF runs). No bank-wide clear.
            for c in range(4):
                et, slot = ets[c // 2], c % 2
                nc.tensor.matmul(
                    ps[32 * c : 32 * c + 2, :],
                    s_rm[0:128, 0:2],
                    et[0:128, 512 * slot : 512 * (slot + 1)],
                    start=True, stop=False, skip_group_check=True,
                    tile_position=(0, 32 * c),
                )
            for half in range(2):
                if DBG_NO_C1Q:
                    break
                for c in range(4):
                    et, slot = ets[c // 2], c % 2
                    nc.tensor.matmul(
                        ps[32 * c : 32 * c + 2, 256 * half : 256 * (half + 1)],
                        s_rm[0:128, 2 + 2 * half : 4 + 2 * half],
                        et[0:128, 1024 + 256 * slot : 1280 + 256 * slot],
                        start=False, stop=(half == 1 and c == 3),
                        skip_group_check=True, tile_position=(0, 32 * c),
                    )
            # drain: one DVE psum->sbuf image copy, then two stride-32
            # partition-gather DMAs into the row-major stats tile.
            img = imgp.tile([128, 512], F32, tag="img")
            if not DBG_NO_IMG:
                nc.vector.tensor_copy(img, ps[:, :])
            if not DBG_NO_GATHER:
                imv = img.rearrange("(c q) f -> c q f", q=32)
                b, qq = divmod(q, 8)
                nc.gpsimd.dma_start(out=s_st0[b][4 * qq : 4 * qq + 4, :],
                                    in_=imv[:, 0, :])
                nc.gpsimd.dma_start(out=s_st1[b][4 * qq : 4 * qq + 4, :],
                                    in_=imv[:, 1, :])

        oo0 = [fin.tile([32, 512], F32, name=f"oo0{b}", tag=f"oo0{b}") for b in range(2)]
        oo1 = [fin.tile([32, 512], F32, name=f"oo1{b}", tag=f"oo1{b}") for b in range(2)]
        rec = [fin.tile([32, 512], F32, name=f"rec{b}", tag=f"rec{b}") for b in range(2)]
        oov = [fin.tile([32, 512], F32, name=f"oov{b}", tag=f"oov{b}") for b in range(2)]
        om = [fin.tile([32, 128, 4], F32, name=f"om{b}", tag=f"om{b}") for b in range(2)]
        op_v = outp.rearrange("(t r) w -> t r w", r=4)

        def finalize_block(rb):
            # rows o = 32*rb .. 32*rb+31 (two r-phases), all tiles base-0
            # -> output rows 2rb::4 and 2rb+1::4
            e192src = s_e192[0:32, :] if rb == 0 else s_e192b
            nc.vector.scalar_tensor_tensor(out=oo0[rb], in0=e192src,
                                           scalar=RV0, in1=s_st0[rb],
                                           op0=mult, op1=add)
            nc.vector.scalar_tensor_tensor(out=oo1[rb], in0=e192src,
                                           scalar=RV1, in1=s_st1[rb],
                                           op0=mult, op1=add)
            nc.vector.reciprocal_approx_fast(out=rec[rb], in_=oo0[rb])
            nc.vector.tensor_mul(oov[rb], oo1[rb], rec[rb])
            nc.vector.tensor_copy(
                om[rb].rearrange("o s q -> o q s"),
                oov[rb].rearrange("o (q s) -> o q s", q=4),
            )
            for k in range(2):
                nc.sync.dma_start(
                    out=op_v[:, 2 * rb + k, :],
                    in_=om[rb].rearrange("o s q -> o (s q)")[16 * k : 16 * (k + 1)],
                )

        granules = [(r, tp) for r in range(4) for tp in range(0, 16, 2)]
        pend = []   # e-tile pairs awaiting stats, in granule order
        nquad = 0
        fins = 0
        for gi, (r, tp) in enumerate(granules):
            pv = pvol.tile([128, 1536], F32, tag="pv")
            et = epool.tile([128, 1536], F16, tag="e")
            l0 = tp + (1 if r >= 2 else 0)
            emit_vol(pv, r, l0, 0)
            if gi == 0:
                # x47 plane: vol192 borrows the first stats-bank buffer
                # (quad 1 reuses it long after e192 is read)
                pv192 = pstat.tile([128, 512], F32, tag="ps")
                nc.tensor.matmul(pv192[0:64, :], s_hm[:, :],
                                 s_xw47.rearrange("p q s -> p (q s)"),
                                 start=True, stop=True)
            nc.scalar.activation(et[:, :], pv[:, :], exp_fn, scale=-1.0)
            if gi == 0:
                nc.scalar.activation(s_e192, pv192[0:64, :],
                                     exp_fn, scale=-1.0)
                # block 1's x47 rows shifted to a base-0 tile via DMA
                nc.gpsimd.dma_start(out=s_e192b, in_=s_e192[32:64, :])
            pend.append(et)
            if len(pend) >= 4 and gi % 2 == 1:
                if not DBG_NO_QUADS:
                    emit_quad(pend[0:2], nquad)
                pend = pend[2:]
                nquad += 1

        while pend:
            if not DBG_NO_QUADS:
                emit_quad(pend[0:2], nquad)
            pend = pend[2:]
            nquad += 1
        while fins < 2:
            finalize_block(fins)
            fins += 1

    nc.compile()
    return nc


_CACHE: dict = {}


def _shard_inputs(x: np.ndarray):
    """Edge-pad and slice per-core shards (memory movement only)."""
    f16 = mybir.dt.np(F16)
    xpad = np.pad(x[:, 0], ((0, 0), (1, 1), (1, 3), (1, 1)), mode="edge")
    amat, rmat, hmat = _build_consts()
    in_maps = []
    for c in range(NCORES):
        b, q = divmod(c, 4)
        xs = xpad[b][:, 16 * q : 16 * q + 20, :]          # [50, 20, 130]
        xsd = np.concatenate([xs[:, 0:19, :], xs[:, 1:20, :]], axis=0)
        xsd = np.ascontiguousarray(xsd.reshape(2 * KP, NROW * 130)).astype(f16)
        x47 = np.ascontiguousarray(xpad[b][48, 16 * q : 16 * q + 18, :]).astype(f16)
        in_maps.append({"xsd": xsd, "x47": x47, "amat": amat, "rmat": rmat,
                        "hmat": hmat})
    return in_maps


def kernel(x: np.ndarray, _trace: bool = False, _tmpdir=None):
    x = np.asarray(x, dtype=np.float32)
    assert x.shape == (2, 1, 48, 64, 128), x.shape
    if "nc" not in _CACHE:
        _CACHE["nc"] = _build_nc()
    nc = _CACHE["nc"]
    in_maps = _shard_inputs(x)
    res = run_bass_kernel_spmd(
        nc, in_maps, list(range(NCORES)), trace=_trace, tmpdir=_tmpdir
    )
    out = np.zeros((2, 256, 512), dtype=np.float32)
    for c in range(NCORES):
        b, q = divmod(c, 4)
        out[b, 64 * q : 64 * (q + 1), :] = res.results[c]["out"]
    if _trace:
        return out, res
    return out
